# revision 1
# baseline (speedup 1.0000x reference)
"""BayesianLinear forward on 8 Trainium2 NeuronCores.

y = x @ W^T + b with W = w_mu + softplus(w_rho) * eps_w,
                     b = b_mu + softplus(b_rho) * eps_b.

Sharding: column-parallel (output features / 8). Each core samples its
weight shard on-chip and computes y^T[o_shard, :] = W_shard @ x^T.

Layout trick: everything is fed pre-transposed from the host so that the
contraction dim (in_features) lands on SBUF partitions with natural,
contiguous DMA patterns. The kernel emits y^T; the host transposes back.
"""

import numpy as np

# Problem shape (hardcoded per contest rules; kernel.py must be self-contained).
IN_F = 4096
OUT_F = 4096
N_TOK = 4096
N_CORES = 8
O_SHARD = OUT_F // N_CORES  # 512 output features per core

P = 128                     # SBUF partitions
KT = IN_F // P              # 32 contraction tiles
MS = O_SHARD // P           # 4 output-feature subtiles per core
N_TILE = 512                # moving-operand tile (fp32 PSUM bank limit)
NT = N_TOK // N_TILE        # 8 token tiles

_CACHE = {}


def _pin_act_table(bacc, mybir):
    """Keep Exp and Ln only in the one ACT table that has both, so the
    compiler never inserts per-op table reloads (1.3 us each, and they sit
    on the weight-sampling critical path)."""
    if getattr(bacc.get_activation_tables, "_pinned", False):
        return
    orig = bacc.get_activation_tables
    EXP = mybir.ActivationFunctionType.Exp
    LN = mybir.ActivationFunctionType.Ln

    def pinned(arch):
        tables = orig(arch)
        for name, funcs in tables.items():
            if name != "natural_log_exp_and_others":
                funcs.discard(EXP)
                funcs.discard(LN)
        return tables

    pinned._pinned = True
    bacc.get_activation_tables = pinned


def _build_nc():
    import concourse.bass as bass  # noqa: F401
    from concourse import bacc, mybir
    from concourse.tile import TileContext

    _pin_act_table(bacc, mybir)

    f32 = mybir.dt.float32
    f32r = mybir.dt.float32r
    AF = mybir.ActivationFunctionType

    nc = bacc.Bacc("TRN2", target_bir_lowering=False, debug=False,
                   num_devices=N_CORES)

    x_t = nc.dram_tensor("x_t", [IN_F, N_TOK], f32r, kind="ExternalInput")
    w_mu_t = nc.dram_tensor("w_mu_t", [IN_F, O_SHARD], f32, kind="ExternalInput")
    w_rho_t = nc.dram_tensor("w_rho_t", [IN_F, O_SHARD], f32, kind="ExternalInput")
    eps_w_t = nc.dram_tensor("eps_w_t", [IN_F, O_SHARD], f32, kind="ExternalInput")
    b_mu = nc.dram_tensor("b_mu", [O_SHARD], f32, kind="ExternalInput")
    b_rho = nc.dram_tensor("b_rho", [O_SHARD], f32, kind="ExternalInput")
    eps_b = nc.dram_tensor("eps_b", [O_SHARD], f32, kind="ExternalInput")
    y_t = nc.dram_tensor("y_t", [O_SHARD, N_TOK], f32, kind="ExternalOutput")

    # W is sampled in k-chunks and consumed wave-by-wave: wave c runs every
    # token tile's chunk-c matmuls (partial sums drained to SBUF
    # accumulators), while chunk c+1's params stream in and get sampled.
    # This spreads the 24 MiB of param DMA across the whole run instead of
    # front-loading it, keeping both DMA and PE near-continuously busy. A
    # chunk is dead after its wave, so W lives in a 2-slot rotating buffer.
    CHUNKS = [2, 6, 8, 8, 8]
    CHMAX = max(CHUNKS)
    PIECE = 4               # x-load granularity (k-tiles per dma_start)
    LOOKAHEAD = 5           # x pieces prefetched ahead of the PE

    with TileContext(nc) as tc:
        with (
            tc.tile_pool(name="wpool", bufs=1) as wpool,
            tc.tile_pool(name="ppool", bufs=1) as ppool,
            tc.tile_pool(name="spool", bufs=1) as spool,
            tc.tile_pool(name="bpool", bufs=1) as bpool,
            tc.tile_pool(name="xpool", bufs=6) as xpool,
            tc.tile_pool(name="opool", bufs=4) as opool,
            tc.tile_pool(name="apool", bufs=1) as apool,
            tc.tile_pool(name="psum", bufs=8, space="PSUM") as psum,
        ):
            # ---- bias vector: b = b_mu + softplus(b_rho) * eps_b ----
            # laid out [P, MS]: partition p of output subtile ms holds
            # b[ms*128 + p].
            bmu_sb = bpool.tile([P, MS], f32, tag="bmu")
            brho_sb = bpool.tile([P, MS], f32, tag="brho")
            beps_sb = bpool.tile([P, MS], f32, tag="beps")
            bvec = bpool.tile([P, MS], f32, tag="bvec")

            def compute_bias():
                nc.scalar.dma_start(bmu_sb[:],
                                    b_mu.rearrange("(s p) -> p s", p=P))
                nc.scalar.dma_start(brho_sb[:],
                                    b_rho.rearrange("(s p) -> p s", p=P))
                nc.scalar.dma_start(beps_sb[:],
                                    eps_b.rearrange("(s p) -> p s", p=P))
                # softplus(r) = ln(1 + exp(r)); Exp/Ln share one ACT table.
                nc.scalar.activation(bvec[:], brho_sb[:], AF.Exp)
                nc.vector.tensor_scalar_add(bvec[:], bvec[:], 1.0)
                nc.scalar.activation(bvec[:], bvec[:], AF.Ln)
                nc.vector.tensor_mul(bvec[:], bvec[:], beps_sb[:])
                nc.vector.tensor_add(bvec[:], bvec[:], bmu_sb[:])

            # 2-slot rotating W buffer; float32r so the sampling's final DVE
            # add rounds W to the PE's fast-fp32 format.
            w_rot = wpool.tile([P, 2, CHMAX, O_SHARD], f32r, tag="w")
            # per-token-tile fp32 partial-sum accumulators
            accs = [apool.tile([P, MS, N_TILE], f32, tag=f"acc{nt}",
                               name=f"acc_{nt}") for nt in range(NT)]
            # x^T column slices, rounded to fp32r on the host. Loaded in
            # PIECE-k-tile batches (few dma_starts, small wait granularity),
            # emitted LOOKAHEAD pieces ahead of the PE so wave boundaries
            # never wait on an x transfer.
            chunk_start = [sum(CHUNKS[:c]) for c in range(len(CHUNKS))]
            x_order = []        # (c, nt, kt_abs, len) in execution order
            for c, CH in enumerate(CHUNKS):
                for nt in range(NT):
                    j = 0
                    while j < CH:
                        L = min(PIECE, CH - j)
                        x_order.append((c, nt, chunk_start[c] + j, L))
                        j += L
            x_tiles = {}
            x_emitted = [0]

            def emit_x_upto(i):
                while x_emitted[0] <= min(i, len(x_order) - 1):
                    c, nt, kt_abs, L = x_order[x_emitted[0]]
                    xt = xpool.tile([P, L, N_TILE], f32r, tag="x",
                                    name=f"xt_{nt}_{kt_abs}",
                                    padded_shape=[P, PIECE, N_TILE])
                    nc.sync.dma_start(
                        xt[:], x_t[kt_abs * P:(kt_abs + L) * P,
                                   nt * N_TILE:(nt + 1) * N_TILE]
                        .rearrange("(j p) n -> p j n", p=P))
                    x_tiles[(nt, kt_abs)] = xt
                    x_emitted[0] += 1

            def load_params(c):
                CH = CHUNKS[c]
                kt0 = chunk_start[c]
                rows = slice(kt0 * P, (kt0 + CH) * P)
                s = spool.tile([P, CH, O_SHARD], f32, tag="s", name=f"s_{c}",
                               padded_shape=[P, CHMAX, O_SHARD])
                mu = ppool.tile([P, CH, O_SHARD], f32, tag="mu",
                                name=f"mu_{c}",
                                padded_shape=[P, CHMAX, O_SHARD])
                eps = ppool.tile([P, CH, O_SHARD], f32, tag="eps",
                                 name=f"eps_{c}",
                                 padded_shape=[P, CHMAX, O_SHARD])
                nc.sync.dma_start(
                    s[:], w_rho_t[rows, :].rearrange("(j p) o -> p j o", p=P))
                nc.sync.dma_start(
                    mu[:], w_mu_t[rows, :].rearrange("(j p) o -> p j o", p=P))
                nc.sync.dma_start(
                    eps[:], eps_w_t[rows, :].rearrange("(j p) o -> p j o", p=P))
                return s, mu, eps

            def sample_piece(c, parts, lo, hi):
                # small pieces so these DVE ops never delay PSUM drains
                # (the in-order DVE would otherwise stall bank releases)
                s, mu, eps = parts
                sl = slice(lo, hi)
                nc.scalar.activation(s[:, sl, :], s[:, sl, :], AF.Exp)
                nc.vector.tensor_scalar_add(s[:, sl, :], s[:, sl, :], 1.0)
                nc.scalar.activation(s[:, sl, :], s[:, sl, :], AF.Ln)
                nc.vector.tensor_mul(s[:, sl, :], s[:, sl, :], eps[:, sl, :])
                nc.vector.tensor_add(w_rot[:, c % 2, sl, :], s[:, sl, :],
                                     mu[:, sl, :])

            def sample_compute(c, parts):
                for lo in range(0, CHUNKS[c], 2):
                    sample_piece(c, parts, lo, min(lo + 2, CHUNKS[c]))

            sample_compute(0, load_params(0))
            compute_bias()
            gpi = 0  # global x-piece index, tracks x_order
            for c, CH in enumerate(CHUNKS):
                next_parts = None
                have_next = c + 1 < len(CHUNKS)
                if have_next and c == 0:
                    next_parts = load_params(c + 1)  # wave 0 is short
                kt0 = chunk_start[c]
                last = c == len(CHUNKS) - 1
                p0 = 1 if c == 0 else 2  # first nt that samples a piece
                for nt in range(NT):
                    if have_next and c > 0 and nt == 1:
                        # issue param DMAs mid-wave so they never sit ahead
                        # of this wave's x pieces in the DMA queue
                        next_parts = load_params(c + 1)
                    if p0 <= nt <= p0 + 3 and next_parts is not None:
                        # spread the next chunk's sampling across the wave,
                        # one piece per token tile
                        CHn = CHUNKS[c + 1]
                        step = (CHn + 3) // 4
                        lo = (nt - p0) * step
                        hi = min(lo + step, CHn)
                        if lo < hi:
                            sample_piece(c + 1, next_parts, lo, hi)
                    ps = [psum.tile([P, N_TILE], f32, tag="ps",
                                    name=f"ps_{nt}_{c}_{i}")
                          for i in range(MS)]
                    j = 0
                    while j < CH:
                        L = min(PIECE, CH - j)
                        emit_x_upto(gpi + LOOKAHEAD)
                        gpi += 1
                        xt = x_tiles.pop((nt, kt0 + j))
                        for jj in range(L):
                            for ms in range(MS):
                                nc.tensor.matmul(
                                    ps[ms][:],
                                    lhsT=w_rot[:, c % 2, j + jj,
                                               ms * P:(ms + 1) * P],
                                    rhs=xt[:, jj, :],
                                    start=(j + jj == 0),
                                    stop=(j + jj == CH - 1),
                                )
                        j += L
                    nsl = slice(nt * N_TILE, (nt + 1) * N_TILE)
                    for ms in range(MS):
                        if c == 0:
                            # wave-0 drain folds the bias in: acc = psum + b
                            nc.vector.tensor_scalar_add(
                                accs[nt][:, ms, :], ps[ms][:],
                                bvec[:, ms:ms + 1])
                        elif not last:
                            nc.vector.tensor_add(accs[nt][:, ms, :],
                                                 accs[nt][:, ms, :], ps[ms][:])
                        else:
                            ot = opool.tile([P, N_TILE], f32, tag="o",
                                            name=f"of_{nt}_{ms}")
                            nc.vector.tensor_add(ot[:], accs[nt][:, ms, :],
                                                 ps[ms][:])
                            nc.gpsimd.dma_start(
                                y_t[ms * P:(ms + 1) * P, nsl], ot[:])

    nc.compile()
    return nc


def _get_nc():
    if "nc" not in _CACHE:
        _CACHE["nc"] = _build_nc()
    return _CACHE["nc"]


def _round_fp32r(a):
    """Round-to-nearest-even into the PE's fp32r format (1s+8e+11m in the
    top 20 bits of the f32 word); the BIR verifier requires fp32r matmul
    operands to be pre-rounded."""
    u = a.view(np.uint32)
    r = (u + np.uint32(0x7FF) + ((u >> np.uint32(12)) & np.uint32(1))) \
        & np.uint32(0xFFFFF000)
    return r.view(np.float32)


def _in_maps(inputs):
    x = np.ascontiguousarray(np.asarray(inputs["x"], dtype=np.float32))
    w_mu = np.asarray(inputs["w_mu"], dtype=np.float32)
    w_rho = np.asarray(inputs["w_rho"], dtype=np.float32)
    eps_w = np.asarray(inputs["eps_w"], dtype=np.float32)
    b_mu = np.asarray(inputs["b_mu"], dtype=np.float32)
    b_rho = np.asarray(inputs["b_rho"], dtype=np.float32)
    eps_b = np.asarray(inputs["eps_b"], dtype=np.float32)

    x_t = _round_fp32r(np.ascontiguousarray(x.T))
    maps = []
    for c in range(N_CORES):
        sl = slice(c * O_SHARD, (c + 1) * O_SHARD)
        maps.append({
            "x_t": x_t,
            "w_mu_t": np.ascontiguousarray(w_mu[sl].T),
            "w_rho_t": np.ascontiguousarray(w_rho[sl].T),
            "eps_w_t": np.ascontiguousarray(eps_w[sl].T),
            "b_mu": np.ascontiguousarray(b_mu[sl]),
            "b_rho": np.ascontiguousarray(b_rho[sl]),
            "eps_b": np.ascontiguousarray(eps_b[sl]),
        })
    return maps


def run(inputs, trace=False, **kwargs):
    """Run on hardware; returns (y [N_TOK, OUT_F], BassKernelResults)."""
    from concourse.bass_utils import run_bass_kernel_spmd

    nc = _get_nc()
    res = run_bass_kernel_spmd(nc, _in_maps(inputs), list(range(N_CORES)),
                               trace=trace, **kwargs)
    y_t = np.concatenate([r["y_t"] for r in res.results], axis=0)
    return np.ascontiguousarray(y_t.T), res


def kernel(**inputs) -> np.ndarray:
    y, _ = run(inputs, trace=False)
    return y



# revision 4
# speedup vs baseline: 1.2683x; 1.2683x over previous
"""BayesianLinear forward on 8 Trainium2 NeuronCores.

y = x @ W^T + b with W = w_mu + softplus(w_rho) * eps_w,
                     b = b_mu + softplus(b_rho) * eps_b.

Sharding: column-parallel (output features / 8). Each core samples its
weight shard on-chip and computes y^T[o_shard, :] = W_shard @ x^T.

Datapath: bf16. x is cast to bf16 on the host (pure dtype/layout prep,
like the pre-transposes); mu/rho/eps stream in as bf16 and the weight
sample W = mu + ln(1+exp(rho))*eps is computed on-chip (ACT exp/ln in
fp32 internals, DVE mul/add), rounding to a bf16 W held fully resident
in SBUF (4 MiB). This halves HBM traffic vs fp32 (52 MiB vs 96 MiB per
core) and enables the PE's fast-weight-load path, so the kernel runs at
the bf16 PE roofline instead of the DMA roofline. PSUM accumulates all
32 k-tiles per (token-tile, out-subtile) group; one bias-fused DVE
drain per group replaces the old 5-wave partial-sum machinery.

Schedule: param chunks (3 x 512 KiB, 8 chunks) stream on the ACT HWDGE
ring while x pieces stream on the SP ring. Token tiles 0 and 1 run
chunk-major in lockstep with sampling (their PE work covers exactly the
~55 us the 20 MiB of phase-1 DMA needs); token tiles 2..7 then run
k-contiguous at full PE rate with x double-buffered ahead.
"""

import numpy as np

# Problem shape (hardcoded per contest rules; kernel.py must be self-contained).
IN_F = 4096
OUT_F = 4096
N_TOK = 4096
N_CORES = 8
O_SHARD = OUT_F // N_CORES  # 512 output features per core

P = 128                     # SBUF partitions
KT = IN_F // P              # 32 contraction k-tiles
MS = O_SHARD // P           # 4 output-feature subtiles per core
N_TILE = 512                # moving-operand tile (fp32 PSUM bank limit)
NT = N_TOK // N_TILE        # 8 token tiles
CH = 4                      # k-tiles per param/sampling chunk
NCH = KT // CH              # 8 chunks
LOOKAHEAD = 6               # x pieces prefetched ahead of the PE

_CACHE = {}


def _pin_act_table(bacc, mybir):
    """Keep Exp and Ln only in the one ACT table that has both, so the
    compiler never inserts per-op table reloads (2.7 us each, and they sit
    on the weight-sampling critical path)."""
    if getattr(bacc.get_activation_tables, "_pinned", False):
        return
    orig = bacc.get_activation_tables
    EXP = mybir.ActivationFunctionType.Exp
    LN = mybir.ActivationFunctionType.Ln

    def pinned(arch):
        tables = orig(arch)
        for name, funcs in tables.items():
            if name != "natural_log_exp_and_others":
                funcs.discard(EXP)
                funcs.discard(LN)
        return tables

    pinned._pinned = True
    bacc.get_activation_tables = pinned


def _build_nc():
    import concourse.bass as bass  # noqa: F401
    from concourse import bacc, mybir
    from concourse.tile import TileContext

    _pin_act_table(bacc, mybir)

    f32 = mybir.dt.float32
    bf16 = mybir.dt.bfloat16
    AF = mybir.ActivationFunctionType

    nc = bacc.Bacc("TRN2", target_bir_lowering=False, debug=False,
                   num_devices=N_CORES)

    x_t = nc.dram_tensor("x_t", [IN_F, N_TOK], bf16, kind="ExternalInput")
    w_mu_t = nc.dram_tensor("w_mu_t", [IN_F, O_SHARD], bf16, kind="ExternalInput")
    w_rho_t = nc.dram_tensor("w_rho_t", [IN_F, O_SHARD], bf16, kind="ExternalInput")
    eps_w_t = nc.dram_tensor("eps_w_t", [IN_F, O_SHARD], bf16, kind="ExternalInput")
    b_mu = nc.dram_tensor("b_mu", [O_SHARD], f32, kind="ExternalInput")
    b_rho = nc.dram_tensor("b_rho", [O_SHARD], f32, kind="ExternalInput")
    eps_b = nc.dram_tensor("eps_b", [O_SHARD], f32, kind="ExternalInput")
    y_t = nc.dram_tensor("y_t", [O_SHARD, N_TOK], f32, kind="ExternalOutput")

    with TileContext(nc) as tc:
        with (
            tc.tile_pool(name="wpool", bufs=1) as wpool,
            tc.tile_pool(name="ppool", bufs=2) as ppool,
            tc.tile_pool(name="bpool", bufs=1) as bpool,
            tc.tile_pool(name="xpool", bufs=10) as xpool,
            tc.tile_pool(name="opool", bufs=8) as opool,
            tc.tile_pool(name="psum", bufs=8, space="PSUM") as psum,
        ):
            # ---- bias vector: b = b_mu + softplus(b_rho) * eps_b ----
            # laid out [P, MS]: partition p of output subtile ms holds
            # b[ms*128 + p].
            bmu_sb = bpool.tile([P, MS], f32, tag="bmu")
            brho_sb = bpool.tile([P, MS], f32, tag="brho")
            beps_sb = bpool.tile([P, MS], f32, tag="beps")
            bvec = bpool.tile([P, MS], f32, tag="bvec")

            def compute_bias():
                nc.scalar.dma_start(bmu_sb[:],
                                    b_mu.rearrange("(s p) -> p s", p=P))
                nc.scalar.dma_start(brho_sb[:],
                                    b_rho.rearrange("(s p) -> p s", p=P))
                nc.scalar.dma_start(beps_sb[:],
                                    eps_b.rearrange("(s p) -> p s", p=P))
                # softplus(r) = ln(1 + exp(r)); Exp/Ln share one ACT table.
                nc.scalar.activation(bvec[:], brho_sb[:], AF.Exp)
                nc.scalar.activation(bvec[:], bvec[:], AF.Ln, bias=1.0)
                nc.vector.tensor_mul(bvec[:], bvec[:], beps_sb[:])
                nc.vector.tensor_add(bvec[:], bvec[:], bmu_sb[:])

            # W shard, fully resident in bf16 (32 KiB/partition).
            w_sb = wpool.tile([P, KT, O_SHARD], bf16, tag="w")

            def load_params(c):
                rows = slice(c * CH * P, (c + 1) * CH * P)
                rho = ppool.tile([P, CH, O_SHARD], bf16, tag="rho",
                                 name=f"rho_{c}")
                mu = ppool.tile([P, CH, O_SHARD], bf16, tag="mu",
                                name=f"mu_{c}")
                eps = ppool.tile([P, CH, O_SHARD], bf16, tag="eps",
                                 name=f"eps_{c}")
                nc.scalar.dma_start(
                    rho[:], w_rho_t[rows, :].rearrange("(j p) o -> p j o", p=P))
                nc.scalar.dma_start(
                    eps[:], eps_w_t[rows, :].rearrange("(j p) o -> p j o", p=P))
                nc.scalar.dma_start(
                    mu[:], w_mu_t[rows, :].rearrange("(j p) o -> p j o", p=P))
                return rho, mu, eps

            def sample(c, parts):
                rho, mu, eps = parts
                sl = slice(c * CH, (c + 1) * CH)
                # sigma = ln(1 + exp(rho)), computed in-place in the rho
                # tile (ACT evaluates in fp32 internally; bf16 storage of
                # exp(rho) costs <0.04% on sigma).
                nc.scalar.activation(rho[:], rho[:], AF.Exp)
                nc.scalar.activation(rho[:], rho[:], AF.Ln, bias=1.0)
                nc.vector.tensor_mul(eps[:], eps[:], rho[:])
                nc.vector.tensor_add(w_sb[:, sl, :], eps[:], mu[:])

            # x^T pieces, [P, CH, N_TILE] bf16 (512 KiB), in PE consumption
            # order with a small emission lookahead so the SP DMA ring stays
            # just ahead of the matmuls without hoarding bandwidth.
            x_order = [(nt, c) for c in range(NCH) for nt in (0, 1)] + \
                      [(nt, c) for nt in range(2, NT) for c in range(NCH)]
            x_tiles = {}
            x_emitted = [0]

            def emit_x_upto(i):
                while x_emitted[0] <= min(i, len(x_order) - 1):
                    nt, c = x_order[x_emitted[0]]
                    xt = xpool.tile([P, CH, N_TILE], bf16, tag="x",
                                    name=f"xt_{nt}_{c}")
                    nc.sync.dma_start(
                        xt[:], x_t[c * CH * P:(c + 1) * CH * P,
                                   nt * N_TILE:(nt + 1) * N_TILE]
                        .rearrange("(j p) n -> p j n", p=P))
                    x_tiles[(nt, c)] = xt
                    x_emitted[0] += 1

            def mm_chunk(ps, nt, c):
                emit_x_upto(x_order.index((nt, c)) + LOOKAHEAD)
                xt = x_tiles.pop((nt, c))
                for j in range(CH):
                    kt = c * CH + j
                    for ms in range(MS):
                        nc.tensor.matmul(
                            ps[ms][:],
                            lhsT=w_sb[:, kt, ms * P:(ms + 1) * P],
                            rhs=xt[:, j, :],
                            start=(kt == 0),
                            stop=(kt == KT - 1),
                        )

            def drain(ps, nt):
                nsl = slice(nt * N_TILE, (nt + 1) * N_TILE)
                for ms in range(MS):
                    ot = opool.tile([P, N_TILE], f32, tag="o",
                                    name=f"of_{nt}_{ms}")
                    nc.vector.tensor_scalar_add(ot[:], ps[ms][:],
                                                bvec[:, ms:ms + 1])
                    nc.gpsimd.dma_start(y_t[ms * P:(ms + 1) * P, nsl], ot[:])

            def psum_group(nt):
                return [psum.tile([P, N_TILE], f32, tag="ps",
                                  name=f"ps_{nt}_{ms}")
                        for ms in range(MS)]

            compute_bias()
            # Queue all param-chunk DMAs up front on the ACT ring (FIFO, in
            # chunk order); they stream while phase 1 consumes them.
            parts = [load_params(c) for c in range(NCH)]

            # Phase 1: token tiles 0 and 1 chunk-major, in lockstep with
            # sampling; all 8 PSUM banks active.
            ps01 = {nt: psum_group(nt) for nt in (0, 1)}
            for c in range(NCH):
                sample(c, parts[c])
                for nt in (0, 1):
                    mm_chunk(ps01[nt], nt, c)
            for nt in (0, 1):
                drain(ps01[nt], nt)

            # Phase 2: token tiles 2..7 k-contiguous at full PE rate.
            for nt in range(2, NT):
                ps = psum_group(nt)
                for c in range(NCH):
                    mm_chunk(ps, nt, c)
                drain(ps, nt)

    nc.compile()
    return nc


def _get_nc():
    if "nc" not in _CACHE:
        _CACHE["nc"] = _build_nc()
    return _CACHE["nc"]


def _in_maps(inputs):
    import ml_dtypes

    bf16 = ml_dtypes.bfloat16
    x = np.asarray(inputs["x"], dtype=np.float32)
    w_mu = np.asarray(inputs["w_mu"], dtype=np.float32)
    w_rho = np.asarray(inputs["w_rho"], dtype=np.float32)
    eps_w = np.asarray(inputs["eps_w"], dtype=np.float32)
    b_mu = np.asarray(inputs["b_mu"], dtype=np.float32)
    b_rho = np.asarray(inputs["b_rho"], dtype=np.float32)
    eps_b = np.asarray(inputs["eps_b"], dtype=np.float32)

    x_t = np.ascontiguousarray(x.T).astype(bf16)
    maps = []
    for c in range(N_CORES):
        sl = slice(c * O_SHARD, (c + 1) * O_SHARD)
        maps.append({
            "x_t": x_t,
            "w_mu_t": np.ascontiguousarray(w_mu[sl].T).astype(bf16),
            "w_rho_t": np.ascontiguousarray(w_rho[sl].T).astype(bf16),
            "eps_w_t": np.ascontiguousarray(eps_w[sl].T).astype(bf16),
            "b_mu": np.ascontiguousarray(b_mu[sl]),
            "b_rho": np.ascontiguousarray(b_rho[sl]),
            "eps_b": np.ascontiguousarray(eps_b[sl]),
        })
    return maps


def run(inputs, trace=False, **kwargs):
    """Run on hardware; returns (y [N_TOK, OUT_F], BassKernelResults)."""
    from concourse.bass_utils import run_bass_kernel_spmd

    nc = _get_nc()
    res = run_bass_kernel_spmd(nc, _in_maps(inputs), list(range(N_CORES)),
                               trace=trace, **kwargs)
    y_t = np.concatenate([r["y_t"] for r in res.results], axis=0)
    return np.ascontiguousarray(y_t.T), res


def kernel(**inputs) -> np.ndarray:
    y, _ = run(inputs, trace=False)
    return y


# revision 7
# speedup vs baseline: 1.3284x; 1.0474x over previous
"""BayesianLinear forward on 8 Trainium2 NeuronCores.

y = x @ W^T + b with W = w_mu + softplus(w_rho) * eps_w,
                     b = b_mu + softplus(b_rho) * eps_b.

Sharding: column-parallel (output features / 8). Each core samples its
weight shard on-chip and computes y^T[o_shard, :] = W_shard @ x^T.

Datapath: bf16. x is cast to bf16 on the host (pure dtype/layout prep,
like the pre-transposes); rho/eps/mu stream in as one host-interleaved
bf16 tensor and the weight sample W = mu + ln(1+exp(rho))*eps is
computed on-chip (ACT exp/ln, fp32 internals; DVE mul/add), rounding to
a bf16 W held fully resident in SBUF (4 MiB). This halves HBM traffic
vs fp32 (52 MiB vs 96 MiB per core) and enables the PE fast-weight-load
path, so the kernel runs at the bf16 PE roofline, not the DMA roofline.
PSUM accumulates all 32 k-tiles per (token-tile, out-subtile) group;
one bias-fused drain per group (split DVE/ACT) replaces the old 5-wave
partial-sum machinery.

Schedule: ALL input DMA rides the SP HWDGE ring in exact consumption
order (params chunk c, then the x pieces first needed at chunk c) — a
single ring spreads one transfer across all 16 SDMA engines, so it
still gets full HBM bandwidth, and keeping ACTIVATE waits off the ring
engine avoids head-of-line blocking of later DMA triggers. Token tiles
0 and 1 run chunk-major in lockstep with param streaming + sampling
(their PE work covers the ~57 us the 20 MiB of phase-1 DMA needs);
token tiles 2..7 then run k-contiguous at full PE rate with x
prefetched ahead. Bias params ride the idle SWDGE ring.
"""

import numpy as np

# Problem shape (hardcoded per contest rules; kernel.py must be self-contained).
IN_F = 4096
OUT_F = 4096
N_TOK = 4096
N_CORES = 8
O_SHARD = OUT_F // N_CORES  # 512 output features per core

P = 128                     # SBUF partitions
KT = IN_F // P              # 32 contraction k-tiles
MS = O_SHARD // P           # 4 output-feature subtiles per core
N_TILE = 512                # moving-operand tile (fp32 PSUM bank limit)
NT = N_TOK // N_TILE        # 8 token tiles
CH = 4                      # k-tiles per param/sampling chunk
NCH = KT // CH              # 8 chunks
XH = 8                      # k-tiles per x piece (1 MiB DMAs)
NXH = KT // XH              # 4 x pieces per token tile
LOOKAHEAD = 4               # x pieces emitted ahead of the PE

_CACHE = {}


def _pin_act_table(bacc, mybir):
    """Keep Exp and Ln only in the one ACT table that has both, so the
    compiler never inserts per-op table reloads (2.7 us each, and they sit
    on the weight-sampling critical path)."""
    if getattr(bacc.get_activation_tables, "_pinned", False):
        return
    orig = bacc.get_activation_tables
    EXP = mybir.ActivationFunctionType.Exp
    LN = mybir.ActivationFunctionType.Ln

    def pinned(arch):
        tables = orig(arch)
        for name, funcs in tables.items():
            if name != "natural_log_exp_and_others":
                funcs.discard(EXP)
                funcs.discard(LN)
        return tables

    pinned._pinned = True
    bacc.get_activation_tables = pinned


def _build_nc():
    import concourse.bass as bass  # noqa: F401
    from concourse import bacc, mybir
    from concourse.tile import TileContext

    _pin_act_table(bacc, mybir)

    f32 = mybir.dt.float32
    bf16 = mybir.dt.bfloat16
    AF = mybir.ActivationFunctionType

    nc = bacc.Bacc("TRN2", target_bir_lowering=False, debug=False,
                   num_devices=N_CORES)

    x_t = nc.dram_tensor("x_t", [IN_F, N_TOK], bf16, kind="ExternalInput")
    # host-interleaved [in, 3, o_shard]: plane 0 = rho, 1 = eps, 2 = mu
    par_t = nc.dram_tensor("par_t", [IN_F, 3, O_SHARD], bf16,
                           kind="ExternalInput")
    # biases pre-laid-out [P, MS] on the host: row p, col s = b[s*128+p]
    b_mu_t = nc.dram_tensor("b_mu_t", [P, MS], f32, kind="ExternalInput")
    b_rho_t = nc.dram_tensor("b_rho_t", [P, MS], f32, kind="ExternalInput")
    eps_b_t = nc.dram_tensor("eps_b_t", [P, MS], f32, kind="ExternalInput")
    y_t = nc.dram_tensor("y_t", [O_SHARD, N_TOK], f32, kind="ExternalOutput")

    with TileContext(nc) as tc:
        with (
            tc.tile_pool(name="wpool", bufs=1) as wpool,
            tc.tile_pool(name="ppool", bufs=3) as ppool,
            tc.tile_pool(name="spool", bufs=2) as spool,
            tc.tile_pool(name="bpool", bufs=1) as bpool,
            tc.tile_pool(name="xpool", bufs=7) as xpool,
            tc.tile_pool(name="opool", bufs=8) as opool,
            tc.tile_pool(name="psum", bufs=8, space="PSUM") as psum,
        ):
            # ---- bias vector: b = b_mu + softplus(b_rho) * eps_b ----
            bmu_sb = bpool.tile([P, MS], f32, tag="bmu")
            brho_sb = bpool.tile([P, MS], f32, tag="brho")
            beps_sb = bpool.tile([P, MS], f32, tag="beps")
            bvec = bpool.tile([P, MS], f32, tag="bvec")

            def compute_bias():
                nc.gpsimd.dma_start(brho_sb[:], b_rho_t[:, :])
                nc.gpsimd.dma_start(beps_sb[:], eps_b_t[:, :])
                nc.gpsimd.dma_start(bmu_sb[:], b_mu_t[:, :])
                # softplus(r) = ln(1 + exp(r)); Exp/Ln share one ACT table.
                nc.scalar.activation(bvec[:], brho_sb[:], AF.Exp)
                nc.scalar.activation(bvec[:], bvec[:], AF.Ln, bias=1.0)
                nc.vector.tensor_mul(bvec[:], bvec[:], beps_sb[:])
                nc.vector.tensor_add(bvec[:], bvec[:], bmu_sb[:])

            # W shard, fully resident in bf16 (32 KiB/partition).
            w_sb = wpool.tile([P, KT, O_SHARD], bf16, tag="w")

            def load_params(c):
                rows = slice(c * CH * P, (c + 1) * CH * P)
                pt = ppool.tile([P, CH, 3, O_SHARD], bf16, tag="pt",
                                name=f"pt_{c}")
                nc.sync.dma_start(
                    pt[:], par_t[rows, :, :]
                    .rearrange("(j p) t o -> p j t o", p=P))
                return pt

            def sample(c, pt, lo, hi):
                # sigma = ln(1 + exp(rho)) staged in bf16 (ACT computes in
                # fp32 internally; bf16 storage of exp(rho) costs <0.04%
                # on sigma); W slice = sigma * eps + mu.
                s = spool.tile([P, CH, O_SHARD], bf16, tag="s", name=f"s_{c}")
                nc.scalar.activation(s[:, lo:hi, :], pt[:, lo:hi, 0, :],
                                     AF.Exp)
                nc.scalar.activation(s[:, lo:hi, :], s[:, lo:hi, :],
                                     AF.Ln, bias=1.0)
                nc.vector.tensor_mul(s[:, lo:hi, :], s[:, lo:hi, :],
                                     pt[:, lo:hi, 1, :])
                nc.vector.tensor_add(w_sb[:, c * CH + lo:c * CH + hi, :],
                                     s[:, lo:hi, :], pt[:, lo:hi, 2, :])

            # x^T pieces [P, XH, N_TILE] bf16 (1 MiB), in PE consumption
            # order. Phase-1 pieces are emitted inline in the chunk loop so
            # the single ring carries params and x interleaved by need;
            # phase-2 pieces trail via the lookahead counter.
            x_order = [(nt, h) for h in range(NXH) for nt in (0, 1)] + \
                      [(nt, h) for nt in range(2, NT) for h in range(NXH)]
            x_index = {p: i for i, p in enumerate(x_order)}
            x_tiles = {}
            x_emitted = [0]

            def emit_x_upto(i):
                while x_emitted[0] <= min(i, len(x_order) - 1):
                    nt, h = x_order[x_emitted[0]]
                    xt = xpool.tile([P, XH, N_TILE], bf16, tag="x",
                                    name=f"xt_{nt}_{h}")
                    nc.sync.dma_start(
                        xt[:], x_t[h * XH * P:(h + 1) * XH * P,
                                   nt * N_TILE:(nt + 1) * N_TILE]
                        .rearrange("(j p) n -> p j n", p=P))
                    x_tiles[(nt, h)] = xt
                    x_emitted[0] += 1

            def mm_chunk(ps, nt, c):
                h = c // 2
                # phase-1 pieces are ring-ordered by the chunk loop; only
                # phase-2 consumption pulls the emission counter ahead
                emit_x_upto(x_index[(nt, h)] + (LOOKAHEAD if nt >= 2 else 0))
                xt = x_tiles[(nt, h)]
                for j in range(CH):
                    kt = c * CH + j
                    for ms in range(MS):
                        nc.tensor.matmul(
                            ps[ms][:],
                            lhsT=w_sb[:, kt, ms * P:(ms + 1) * P],
                            rhs=xt[:, (c % 2) * CH + j, :],
                            start=(kt == 0),
                            stop=(kt == KT - 1),
                        )
                if c % 2 == 1 and nt == 1:
                    x_tiles.pop((0, h))
                    x_tiles.pop((1, h))
                elif c % 2 == 1 and nt >= 2:
                    x_tiles.pop((nt, h))

            def drain(ps, nt):
                # split PSUM->SBUF bias-fused drains across DVE and ACT so
                # each group's evacuation takes ~2 op-times, not 4.
                nsl = slice(nt * N_TILE, (nt + 1) * N_TILE)
                for ms in range(MS):
                    ot = opool.tile([P, N_TILE], f32, tag="o",
                                    name=f"of_{nt}_{ms}")
                    if ms < 2:
                        nc.vector.tensor_scalar_add(ot[:], ps[ms][:],
                                                    bvec[:, ms:ms + 1])
                    else:
                        nc.scalar.activation(ot[:], ps[ms][:], AF.Identity,
                                             bias=bvec[:, ms:ms + 1])
                    nc.gpsimd.dma_start(y_t[ms * P:(ms + 1) * P, nsl], ot[:])

            def psum_group(nt):
                return [psum.tile([P, N_TILE], f32, tag="ps",
                                  name=f"ps_{nt}_{ms}")
                        for ms in range(MS)]

            compute_bias()

            # Phase 1: token tiles 0 and 1 chunk-major, in lockstep with
            # param streaming + sampling; all 8 PSUM banks active.
            ps01 = {nt: psum_group(nt) for nt in (0, 1)}
            for c in range(NCH):
                pt = load_params(c)
                if c % 2 == 0:
                    emit_x_upto(x_index[(1, c // 2)])
                if c == NCH - 1:
                    # phase-2 head start: first two nt=2 pieces ride the
                    # ring tail behind the last param chunk
                    emit_x_upto(x_index[(2, 1)])
                if c == 0:
                    # halve first-chunk sampling granularity so the first
                    # matmuls start ~2 us earlier
                    sample(c, pt, 0, 2)
                    sample(c, pt, 2, CH)
                else:
                    sample(c, pt, 0, CH)
                for nt in (0, 1):
                    mm_chunk(ps01[nt], nt, c)
            for nt in (0, 1):
                drain(ps01[nt], nt)

            # Phase 2: token tiles 2..7 k-contiguous at full PE rate.
            for nt in range(2, NT):
                ps = psum_group(nt)
                for c in range(NCH):
                    mm_chunk(ps, nt, c)
                drain(ps, nt)

    nc.compile()
    return nc


def _get_nc():
    if "nc" not in _CACHE:
        _CACHE["nc"] = _build_nc()
    return _CACHE["nc"]


def _in_maps(inputs):
    import ml_dtypes

    bf16 = ml_dtypes.bfloat16
    x = np.asarray(inputs["x"], dtype=np.float32)
    w_mu = np.asarray(inputs["w_mu"], dtype=np.float32)
    w_rho = np.asarray(inputs["w_rho"], dtype=np.float32)
    eps_w = np.asarray(inputs["eps_w"], dtype=np.float32)
    b_mu = np.asarray(inputs["b_mu"], dtype=np.float32)
    b_rho = np.asarray(inputs["b_rho"], dtype=np.float32)
    eps_b = np.asarray(inputs["eps_b"], dtype=np.float32)

    x_t = np.ascontiguousarray(x.T).astype(bf16)
    maps = []
    for c in range(N_CORES):
        sl = slice(c * O_SHARD, (c + 1) * O_SHARD)
        par = np.stack([w_rho[sl].T, eps_w[sl].T, w_mu[sl].T],
                       axis=1)  # [IN_F, 3, O_SHARD]
        maps.append({
            "x_t": x_t,
            "par_t": np.ascontiguousarray(par).astype(bf16),
            "b_mu_t": np.ascontiguousarray(b_mu[sl].reshape(MS, P).T),
            "b_rho_t": np.ascontiguousarray(b_rho[sl].reshape(MS, P).T),
            "eps_b_t": np.ascontiguousarray(eps_b[sl].reshape(MS, P).T),
        })
    return maps


def run(inputs, trace=False, **kwargs):
    """Run on hardware; returns (y [N_TOK, OUT_F], BassKernelResults)."""
    from concourse.bass_utils import run_bass_kernel_spmd

    nc = _get_nc()
    res = run_bass_kernel_spmd(nc, _in_maps(inputs), list(range(N_CORES)),
                               trace=trace, **kwargs)
    y_t = np.concatenate([r["y_t"] for r in res.results], axis=0)
    return np.ascontiguousarray(y_t.T), res


def kernel(**inputs) -> np.ndarray:
    y, _ = run(inputs, trace=False)
    return y
